# revision 16
# baseline (speedup 1.0000x reference)
"""Causal multi-head attention (B=4, S=2048, D=1024, H=16, HD=64) on 8 NeuronCores.

Sharding: core c handles batch b=c//2 and head-group hg=c%2 (8 heads each).
Each core computes out^T_partial = Wo_hg^T @ ctx_hg^T for its (b, hg); the host
sums the two head-group partials per batch, transposes, and adds the bias.

All matmuls run in bf16 (fp32 PSUM accumulation); softmax runs without max
subtraction (scores are O(1) here) using exp on the ScalarE and a ones-column
appended to V so the denominators fall out of the ctx matmul for free.
"""

import sys

for _p in ("/opt/trn_rl_repo",):
    if _p not in sys.path:
        sys.path.insert(0, _p)

import numpy as np
import ml_dtypes
from contextlib import ExitStack

import concourse.bacc as bacc
import concourse.tile as tile
from concourse import mybir
from concourse.bass_utils import run_bass_kernel_spmd

F32 = mybir.dt.float32
BF16 = mybir.dt.bfloat16
Exp = mybir.ActivationFunctionType.Exp

B, S, D, H, HD = 4, 2048, 1024, 16, 64
NC = 8          # cores
HL = 8          # heads per core (head-group)
DH = HL * HD    # 512, per-core head dim
KT = D // 128   # 8 k-tiles over d_in
ST = S // 128   # 16 tiles over sequence
NB = S // 512   # 4 q-superblocks
SCALE = 1.0 / np.sqrt(HD)


def _build_nc(debug=False):
    nc = bacc.Bacc("TRN2", target_bir_lowering=False)

    xT = nc.declare_dram_parameter("xT", [D, S], BF16, isOutput=False)
    wq = nc.declare_dram_parameter("wq", [D, DH], BF16, isOutput=False)
    wk = nc.declare_dram_parameter("wk", [D, DH], BF16, isOutput=False)
    wv = nc.declare_dram_parameter("wv", [D, DH], BF16, isOutput=False)
    wo = nc.declare_dram_parameter("wo", [DH, D], BF16, isOutput=False)
    tri = nc.declare_dram_parameter("tri", [128, 128], BF16, isOutput=False)
    outT = nc.declare_dram_parameter("outT", [D, S], F32, isOutput=True)
    if debug:
        d_qT = nc.declare_dram_parameter("d_qT", [DH, S], BF16, isOutput=True)
        d_kT = nc.declare_dram_parameter("d_kT", [DH, S], BF16, isOutput=True)
        d_v = nc.declare_dram_parameter("d_v", [S, HL * (HD + 1)], BF16, isOutput=True)
        d_ctxT = nc.declare_dram_parameter("d_ctxT", [DH, S], BF16, isOutput=True)
        d_cun = nc.declare_dram_parameter("d_cun", [HL * 65, S], F32, isOutput=True)
        d_bcs = nc.declare_dram_parameter("d_bcs", [64, S], BF16, isOutput=True)

    with tile.TileContext(nc) as tc, ExitStack() as ctx:
        const_pool = ctx.enter_context(tc.tile_pool(name="const", bufs=1))
        xT_pool = ctx.enter_context(tc.tile_pool(name="xT", bufs=1))
        w_pool = ctx.enter_context(tc.tile_pool(name="w", bufs=1))
        qk_pool = ctx.enter_context(tc.tile_pool(name="qk", bufs=1))
        v_pool = ctx.enter_context(tc.tile_pool(name="v", bufs=1))
        ctxT_pool = ctx.enter_context(tc.tile_pool(name="ctxT", bufs=1))
        e_pool = ctx.enter_context(tc.tile_pool(name="e", bufs=8))
        r_pool = ctx.enter_context(tc.tile_pool(name="r", bufs=4))
        o_pool = ctx.enter_context(tc.tile_pool(name="o", bufs=2))
        if debug:
            dbg_pool = ctx.enter_context(tc.tile_pool(name="dbg", bufs=1))
        ps_gen = ctx.enter_context(tc.tile_pool(name="ps_gen", bufs=1, space="PSUM"))
        ps_s = ctx.enter_context(tc.tile_pool(name="ps_s", bufs=2, space="PSUM"))
        ps_c = ctx.enter_context(tc.tile_pool(name="ps_c", bufs=1, space="PSUM"))

        # ---- constants ----
        trit = const_pool.tile([128, 128], BF16)
        nc.sync.dma_start(trit[:], tri[:])
        onesb = const_pool.tile([1, 64], BF16)
        nc.vector.memset(onesb[:], 1.0)

        # ---- load x^T and weights ----
        xt = [xT_pool.tile([128, S], BF16, tag=f"xt{_}", name=f"xt{_}") for _ in range(KT)]
        for k in range(KT):
            nc.sync.dma_start(xt[k][:], xT[128 * k : 128 * (k + 1), :])
        wqt = [w_pool.tile([128, DH], BF16, tag=f"wqt{_}", name=f"wqt{_}") for _ in range(KT)]
        wkt = [w_pool.tile([128, DH], BF16, tag=f"wkt{_}", name=f"wkt{_}") for _ in range(KT)]
        wvt = [w_pool.tile([128, DH], BF16, tag=f"wvt{_}", name=f"wvt{_}") for _ in range(KT)]
        for k in range(KT):
            nc.sync.dma_start(wqt[k][:], wq[128 * k : 128 * (k + 1), :])
            nc.sync.dma_start(wkt[k][:], wk[128 * k : 128 * (k + 1), :])
            nc.sync.dma_start(wvt[k][:], wv[128 * k : 128 * (k + 1), :])
        wot = [w_pool.tile([128, D], BF16, tag=f"wot{_}", name=f"wot{_}") for _ in range(DH // 128)]
        for k in range(DH // 128):
            nc.sync.dma_start(wot[k][:], wo[128 * k : 128 * (k + 1), :])

        # ---- phase 1: Q^T, K^T = W^T @ x^T   [DH, S] as 4 tiles [128, S] ----
        qTt = [qk_pool.tile([128, S], BF16, tag=f"qT{_}", name=f"qT{_}") for _ in range(DH // 128)]
        kTt = [qk_pool.tile([128, S], BF16, tag=f"kT{_}", name=f"kT{_}") for _ in range(DH // 128)]
        for wt, dst in ((wqt, qTt), (wkt, kTt)):
            for m in range(DH // 128):
                for np_ in range(NB // 2):
                    psA = ps_gen.tile([128, 512], F32, tag="pgA", name="psA")
                    psB = ps_gen.tile([128, 512], F32, tag="pgB", name="psB")
                    for k in range(KT):
                        lhsT = wt[k][:, 128 * m : 128 * (m + 1)]
                        for n, pst in ((2 * np_, psA), (2 * np_ + 1, psB)):
                            nc.tensor.matmul(
                                pst[:], lhsT, xt[k][:, 512 * n : 512 * (n + 1)],
                                start=(k == 0), stop=(k == KT - 1),
                            )
                    for n, pst in ((2 * np_, psA), (2 * np_ + 1, psB)):
                        nc.vector.tensor_copy(dst[m][:, 512 * n : 512 * (n + 1)], pst[:])

        # ---- phase 1c: V natural [S, DH] as 16 tiles [128, 8*65] (ones col per head) ----
        vt = [v_pool.tile([128, HL * (HD + 1)], BF16, tag=f"v{_}", name=f"v{_}") for _ in range(ST)]
        for st in range(ST):
            nc.vector.memset(vt[st].rearrange("p (h c) -> p h c", c=HD + 1)[:, :, HD], 1.0)
            pv = ps_gen.tile([128, 512], F32, tag=("pgA" if st % 2 == 0 else "pgB"), name="pv")
            for k in range(KT):
                nc.tensor.matmul(
                    pv[:], xt[k][:, 128 * st : 128 * (st + 1)], wvt[k][:],
                    start=(k == 0), stop=(k == KT - 1),
                )
            nc.vector.tensor_copy(
                vt[st].rearrange("p (h c) -> p h c", c=HD + 1)[:, :, 0:HD],
                pv.rearrange("p (h c) -> p h c", c=HD)[:],
            )

        # ---- phase 2: per head attention in S^T layout ----
        ctxT = [ctxT_pool.tile([128, S], BF16, tag=f"ctxT{_}", name=f"ctxT{_}") for _ in range(DH // 128)]
        for h in range(HL):
            m, r = h // 2, h % 2
            qTh = qTt[m][64 * r : 64 * r + 64, :]
            kTh = kTt[m][64 * r : 64 * r + 64, :]
            cps = [ps_c.tile([65, 512], F32, tag=f"pc{_}", name=f"pctx{_}") for _ in range(NB)]
            for j in range(ST):
                Imin = j // 4
                d0 = j - 4 * Imin
                # S^T blocks for this k-tile against all valid q-superblocks
                eb = {}
                for I in range(Imin, NB):
                    d = d0 if I == Imin else -1
                    lo = 128 * d if d > 0 else 0
                    sp = ps_s.tile([128, 512], F32, tag="ps")
                    nc.tensor.matmul(
                        sp[:, lo:512],
                        kTh[:, 128 * j : 128 * (j + 1)],
                        qTh[:, 512 * I + lo : 512 * (I + 1)],
                        start=True, stop=True,
                    )
                    e = e_pool.tile([128, 512], BF16, tag="e")
                    if lo > 0:
                        nc.vector.memset(e[:, 0:lo], 0.0)
                    nc.scalar.activation(e[:, lo:512], sp[:, lo:512], Exp, scale=float(SCALE))
                    if d >= 0:
                        nc.vector.tensor_tensor(
                            e[:, lo : lo + 128], e[:, lo : lo + 128], trit[:],
                            mybir.AluOpType.mult,
                        )
                    eb[I] = e
                vh = vt[j][:, (HD + 1) * h : (HD + 1) * (h + 1)]
                for I in range(Imin, NB):
                    nc.tensor.matmul(
                        cps[I][:], vh, eb[I][:],
                        start=(j == 0), stop=(j == 4 * I + 3),
                    )
                if j % 4 == 3:
                    # cps[Imin] is complete: normalize it now
                    I = Imin
                    cun = r_pool.tile([65, 512], F32, tag="cun", name="cun")
                    nc.vector.tensor_copy(cun[:], cps[I][:])
                    den0 = r_pool.tile([1, 512], F32, tag="den0", name="den0")
                    nc.sync.dma_start(den0[0:1, :], cun[64:65, :])
                    rec0 = r_pool.tile([1, 512], F32, tag="rec0", name="rec0")
                    nc.vector.reciprocal_approx_fast(rec0[0:1, :], den0[0:1, :])
                    recb = r_pool.tile([1, 512], BF16, tag="recb", name="recb")
                    nc.vector.tensor_copy(recb[:], rec0[:])
                    bc = ps_s.tile([128, 512], F32, tag="ps", name="bc")[0:64, :]
                    nc.tensor.matmul(
                        bc[:], onesb[0:1, 0:64], recb[0:1, :], start=True, stop=True
                    )
                    nrm = r_pool.tile([64, 512], BF16, tag="nrm", name="nrm")
                    nc.vector.tensor_tensor(nrm[:], cun[0:64, :], bc[:], mybir.AluOpType.mult)
                    nc.sync.dma_start(
                        ctxT[m][64 * r : 64 * r + 64, 512 * I : 512 * (I + 1)], nrm[:]
                    )
            if debug:
                for I in range(NB):
                    dcu = dbg_pool.tile([65, 512], F32, tag="dcu", name="dcu")
                    nc.vector.tensor_copy(dcu[:], cps[I][:])
                    nc.sync.dma_start(d_cun[65 * h : 65 * (h + 1), 512 * I : 512 * (I + 1)], dcu[:])


        if debug:
            for m in range(DH // 128):
                nc.sync.dma_start(d_qT[128 * m : 128 * (m + 1), :], qTt[m][:])
                nc.sync.dma_start(d_kT[128 * m : 128 * (m + 1), :], kTt[m][:])
                nc.sync.dma_start(d_ctxT[128 * m : 128 * (m + 1), :], ctxT[m][:])
            for st in range(ST):
                nc.sync.dma_start(d_v[128 * st : 128 * (st + 1), :], vt[st][:])

        # ---- phase 3: out^T = Wo^T @ ctx^T  [D, S] ----
        for m in range(D // 128):
            ot = o_pool.tile([128, S], F32, tag="ot")
            for np_ in range(NB // 2):
                psA = ps_gen.tile([128, 512], F32, tag="pgA", name="poA")
                psB = ps_gen.tile([128, 512], F32, tag="pgB", name="poB")
                for k in range(DH // 128):
                    lhsT = wot[k][:, 128 * m : 128 * (m + 1)]
                    for n, pst in ((2 * np_, psA), (2 * np_ + 1, psB)):
                        nc.tensor.matmul(
                            pst[:], lhsT, ctxT[k][:, 512 * n : 512 * (n + 1)],
                            start=(k == 0), stop=(k == DH // 128 - 1),
                        )
                for n, pst in ((2 * np_, psA), (2 * np_ + 1, psB)):
                    nc.vector.tensor_copy(ot[:, 512 * n : 512 * (n + 1)], pst[:])
            nc.sync.dma_start(outT[128 * m : 128 * (m + 1), :], ot[:])

    nc.compile()
    return nc


_NC_CACHE = None


def kernel(x, Wq, Wk, Wv, Wo, bo):
    global _NC_CACHE
    if _NC_CACHE is None:
        _NC_CACHE = _build_nc()
    nc = _NC_CACHE

    bf = ml_dtypes.bfloat16
    tri = np.triu(np.ones((128, 128), dtype=np.float32)).astype(bf)
    in_maps = []
    for c in range(NC):
        b, hg = c // 2, c % 2
        cols = slice(DH * hg, DH * (hg + 1))
        in_maps.append(
            {
                "xT": np.ascontiguousarray(np.asarray(x)[b].T).astype(bf),
                "wq": np.asarray(Wq)[:, cols].astype(bf),
                "wk": np.asarray(Wk)[:, cols].astype(bf),
                "wv": np.asarray(Wv)[:, cols].astype(bf),
                "wo": np.asarray(Wo)[cols, :].astype(bf),
                "tri": tri,
            }
        )
    res = run_bass_kernel_spmd(nc, in_maps, core_ids=list(range(NC)))
    out = np.empty((B, S, D), dtype=np.float32)
    bo32 = np.asarray(bo, dtype=np.float32)
    for b in range(B):
        acc = res.results[2 * b]["outT"].astype(np.float32) + res.results[2 * b + 1][
            "outT"
        ].astype(np.float32)
        out[b] = acc.T + bo32
    return out


# revision 18
# speedup vs baseline: 1.2409x; 1.2409x over previous
"""Causal multi-head attention (B=4, S=2048, D=1024, H=16, HD=64) on 8 NeuronCores.

Sharding: core c handles batch b=c//2 and head-group hg=c%2 (8 heads each).
Each core computes out^T_partial = Wo_hg^T @ ctx_hg^T for its (b, hg); the host
sums the two head-group partials per batch, transposes, and adds the bias.

All matmuls run in bf16 (fp32 PSUM accumulation); softmax runs without max
subtraction (scores are O(1) here) using exp on the ScalarE and a ones-column
appended to V so the denominators fall out of the ctx matmul for free.
"""

import sys

for _p in ("/opt/trn_rl_repo",):
    if _p not in sys.path:
        sys.path.insert(0, _p)

import numpy as np
import ml_dtypes
from contextlib import ExitStack

import concourse.bacc as bacc
import concourse.tile as tile
from concourse import mybir
from concourse.bass_utils import run_bass_kernel_spmd

F32 = mybir.dt.float32
BF16 = mybir.dt.bfloat16
Exp = mybir.ActivationFunctionType.Exp

B, S, D, H, HD = 4, 2048, 1024, 16, 64
NC = 8          # cores
HL = 8          # heads per core (head-group)
DH = HL * HD    # 512, per-core head dim
KT = D // 128   # 8 k-tiles over d_in
ST = S // 128   # 16 tiles over sequence
NB = S // 512   # 4 q-superblocks
SCALE = 1.0 / np.sqrt(HD)


def _build_nc(debug=False):
    nc = bacc.Bacc("TRN2", target_bir_lowering=False)

    xT = nc.declare_dram_parameter("xT", [D, S], BF16, isOutput=False)
    wq = nc.declare_dram_parameter("wq", [D, DH], BF16, isOutput=False)
    wk = nc.declare_dram_parameter("wk", [D, DH], BF16, isOutput=False)
    wv = nc.declare_dram_parameter("wv", [D, DH], BF16, isOutput=False)
    wo = nc.declare_dram_parameter("wo", [DH, D], BF16, isOutput=False)
    tri = nc.declare_dram_parameter("tri", [128, 128], BF16, isOutput=False)
    outT = nc.declare_dram_parameter("outT", [D, S], F32, isOutput=True)
    if debug:
        d_qT = nc.declare_dram_parameter("d_qT", [DH, S], BF16, isOutput=True)
        d_kT = nc.declare_dram_parameter("d_kT", [DH, S], BF16, isOutput=True)
        d_v = nc.declare_dram_parameter("d_v", [S, HL * (HD + 1)], BF16, isOutput=True)
        d_ctxT = nc.declare_dram_parameter("d_ctxT", [DH, S], BF16, isOutput=True)
        d_cun = nc.declare_dram_parameter("d_cun", [HL * 65, S], F32, isOutput=True)
        d_bcs = nc.declare_dram_parameter("d_bcs", [64, S], BF16, isOutput=True)

    with tile.TileContext(nc) as tc, ExitStack() as ctx:
        const_pool = ctx.enter_context(tc.tile_pool(name="const", bufs=1))
        xT_pool = ctx.enter_context(tc.tile_pool(name="xT", bufs=1))
        w_pool = ctx.enter_context(tc.tile_pool(name="w", bufs=1))
        qk_pool = ctx.enter_context(tc.tile_pool(name="qk", bufs=1))
        v_pool = ctx.enter_context(tc.tile_pool(name="v", bufs=1))
        ctxT_pool = ctx.enter_context(tc.tile_pool(name="ctxT", bufs=1))
        e_pool = ctx.enter_context(tc.tile_pool(name="e", bufs=8))
        r_pool = ctx.enter_context(tc.tile_pool(name="r", bufs=4))
        o_pool = ctx.enter_context(tc.tile_pool(name="o", bufs=2))
        if debug:
            dbg_pool = ctx.enter_context(tc.tile_pool(name="dbg", bufs=1))
        ps_gen = ctx.enter_context(tc.tile_pool(name="ps_gen", bufs=1, space="PSUM"))
        ps_s = ctx.enter_context(tc.tile_pool(name="ps_s", bufs=2, space="PSUM"))
        ps_c = ctx.enter_context(tc.tile_pool(name="ps_c", bufs=1, space="PSUM"))

        # ---- constants ----
        trit = const_pool.tile([128, 128], BF16)
        nc.sync.dma_start(trit[:], tri[:])
        onesb = const_pool.tile([1, 64], BF16)
        nc.vector.memset(onesb[:], 1.0)

        # ---- load x^T and weights ----
        xt = [xT_pool.tile([128, S], BF16, tag=f"xt{_}", name=f"xt{_}") for _ in range(KT)]
        for k in range(KT):
            nc.sync.dma_start(xt[k][:], xT[128 * k : 128 * (k + 1), :])
        wqt = [w_pool.tile([128, DH], BF16, tag=f"wqt{_}", name=f"wqt{_}") for _ in range(KT)]
        wkt = [w_pool.tile([128, DH], BF16, tag=f"wkt{_}", name=f"wkt{_}") for _ in range(KT)]
        wvt = [w_pool.tile([128, DH], BF16, tag=f"wvt{_}", name=f"wvt{_}") for _ in range(KT)]
        for k in range(KT):
            nc.sync.dma_start(wqt[k][:], wq[128 * k : 128 * (k + 1), :])
            nc.sync.dma_start(wkt[k][:], wk[128 * k : 128 * (k + 1), :])
            nc.sync.dma_start(wvt[k][:], wv[128 * k : 128 * (k + 1), :])
        wot = [w_pool.tile([128, D], BF16, tag=f"wot{_}", name=f"wot{_}") for _ in range(DH // 128)]
        for k in range(DH // 128):
            nc.sync.dma_start(wot[k][:], wo[128 * k : 128 * (k + 1), :])

        # ---- phase 1: Q^T, K^T = W^T @ x^T   [DH, S] as 4 tiles [128, S] ----
        qTt = [qk_pool.tile([128, S], BF16, tag=f"qT{_}", name=f"qT{_}") for _ in range(DH // 128)]
        kTt = [qk_pool.tile([128, S], BF16, tag=f"kT{_}", name=f"kT{_}") for _ in range(DH // 128)]
        for wt, dst in ((wqt, qTt), (wkt, kTt)):
            for m in range(DH // 128):
                for np_ in range(NB // 2):
                    psA = ps_gen.tile([128, 512], F32, tag="pgA", name="psA")
                    psB = ps_gen.tile([128, 512], F32, tag="pgB", name="psB")
                    for k in range(KT):
                        lhsT = wt[k][:, 128 * m : 128 * (m + 1)]
                        for n, pst in ((2 * np_, psA), (2 * np_ + 1, psB)):
                            nc.tensor.matmul(
                                pst[:], lhsT, xt[k][:, 512 * n : 512 * (n + 1)],
                                start=(k == 0), stop=(k == KT - 1),
                            )
                    for n, pst in ((2 * np_, psA), (2 * np_ + 1, psB)):
                        nc.vector.tensor_copy(dst[m][:, 512 * n : 512 * (n + 1)], pst[:])

        # ---- phase 1c: V natural [S, DH] as 16 tiles [128, 8*65] (ones col per head) ----
        vt = [v_pool.tile([128, HL * (HD + 1)], BF16, tag=f"v{_}", name=f"v{_}") for _ in range(ST)]
        for st in range(ST):
            nc.vector.memset(vt[st].rearrange("p (h c) -> p h c", c=HD + 1)[:, :, HD], 1.0)
            pv = ps_gen.tile([128, 512], F32, tag=("pgA" if st % 2 == 0 else "pgB"), name="pv")
            for k in range(KT):
                nc.tensor.matmul(
                    pv[:], xt[k][:, 128 * st : 128 * (st + 1)], wvt[k][:],
                    start=(k == 0), stop=(k == KT - 1),
                )
            nc.vector.tensor_copy(
                vt[st].rearrange("p (h c) -> p h c", c=HD + 1)[:, :, 0:HD],
                pv.rearrange("p (h c) -> p h c", c=HD)[:],
            )

        # ---- phase 2: per head attention in S^T layout, q-superblock pairs ----
        ctxT = [ctxT_pool.tile([128, S], BF16, tag=f"ctxT{_}", name=f"ctxT{_}") for _ in range(DH // 128)]
        for h in range(HL):
            m, r = h // 2, h % 2
            qTh = qTt[m][64 * r : 64 * r + 64, :]
            kTh = kTt[m][64 * r : 64 * r + 64, :]
            for ip in range(NB // 2):
                I0 = 2 * ip  # superblocks I0, I0+1 handled this sweep
                cps = [
                    ps_c.tile([65, 512], F32, tag=f"pc{_}", name=f"pctx{_}")
                    for _ in range(2)
                ]
                for j in range(8 * ip + 8):
                    jb = j // 4  # first valid superblock for this k-tile
                    # S^T blocks into one [128,1024] psum (2 banks)
                    sp = ps_s.tile([128, 1024], F32, tag="ps", name="sp")
                    e = e_pool.tile([128, 1024], BF16, tag="e", name="e")
                    valid = []  # (li, lo) li: 0/1 slot, lo: masked-out prefix
                    for I in (I0, I0 + 1):
                        if jb > I:
                            continue
                        li = I - I0
                        lo = 128 * (j - 4 * I) if jb == I else 0
                        valid.append((li, lo))
                        nc.tensor.matmul(
                            sp[:, 512 * li + lo : 512 * (li + 1)],
                            kTh[:, 128 * j : 128 * (j + 1)],
                            qTh[:, 512 * I + lo : 512 * (I + 1)],
                            start=True, stop=True,
                        )
                    li0, lo0 = valid[0]
                    base = 512 * li0 + lo0
                    if lo0 > 0:
                        nc.vector.memset(e[:, 512 * li0 : base], 0.0)
                    nc.scalar.activation(
                        e[:, base : 1024], sp[:, base : 1024], Exp, scale=float(SCALE)
                    )
                    for li, lo in valid:
                        if jb == I0 + li:  # diagonal block: triangular mask
                            nc.vector.tensor_tensor(
                                e[:, 512 * li + lo : 512 * li + lo + 128],
                                e[:, 512 * li + lo : 512 * li + lo + 128],
                                trit[:], mybir.AluOpType.mult,
                            )
                    vh = vt[j][:, (HD + 1) * h : (HD + 1) * (h + 1)]
                    for li, lo in valid:
                        I = I0 + li
                        nc.tensor.matmul(
                            cps[li][:], vh, e[:, 512 * li : 512 * (li + 1)],
                            start=(j == 0), stop=(j == 4 * I + 3),
                        )
                    if j % 4 == 3 and jb >= I0:
                        # cps[jb - I0] is complete: normalize it
                        li = jb - I0
                        I = jb
                        cun = r_pool.tile([65, 512], F32, tag="cun", name="cun")
                        nc.vector.tensor_copy(cun[:], cps[li][:])
                        den0 = r_pool.tile([1, 512], F32, tag="den0", name="den0")
                        nc.sync.dma_start(den0[0:1, :], cun[64:65, :])
                        rec0 = r_pool.tile([1, 512], F32, tag="rec0", name="rec0")
                        nc.vector.reciprocal_approx_fast(rec0[0:1, :], den0[0:1, :])
                        recb = r_pool.tile([1, 512], BF16, tag="recb", name="recb")
                        nc.vector.tensor_copy(recb[:], rec0[:])
                        bc = ps_gen.tile([128, 512], F32, tag="pgA", name="bc")[0:64, :]
                        nc.tensor.matmul(
                            bc[:], onesb[0:1, 0:64], recb[0:1, :], start=True, stop=True
                        )
                        nrm = r_pool.tile([64, 512], BF16, tag="nrm", name="nrm")
                        nc.vector.tensor_tensor(
                            nrm[:], cun[0:64, :], bc[:], mybir.AluOpType.mult
                        )
                        nc.sync.dma_start(
                            ctxT[m][64 * r : 64 * r + 64, 512 * I : 512 * (I + 1)],
                            nrm[:],
                        )

        # ---- phase 3: out^T = Wo^T @ ctx^T  [D, S] ----
        for m in range(D // 128):
            ot = o_pool.tile([128, S], F32, tag="ot")
            for np_ in range(NB // 2):
                psA = ps_gen.tile([128, 512], F32, tag="pgA", name="poA")
                psB = ps_gen.tile([128, 512], F32, tag="pgB", name="poB")
                for k in range(DH // 128):
                    lhsT = wot[k][:, 128 * m : 128 * (m + 1)]
                    for n, pst in ((2 * np_, psA), (2 * np_ + 1, psB)):
                        nc.tensor.matmul(
                            pst[:], lhsT, ctxT[k][:, 512 * n : 512 * (n + 1)],
                            start=(k == 0), stop=(k == DH // 128 - 1),
                        )
                for n, pst in ((2 * np_, psA), (2 * np_ + 1, psB)):
                    nc.vector.tensor_copy(ot[:, 512 * n : 512 * (n + 1)], pst[:])
            nc.sync.dma_start(outT[128 * m : 128 * (m + 1), :], ot[:])

    nc.compile()
    return nc


_NC_CACHE = None


def kernel(x, Wq, Wk, Wv, Wo, bo):
    global _NC_CACHE
    if _NC_CACHE is None:
        _NC_CACHE = _build_nc()
    nc = _NC_CACHE

    bf = ml_dtypes.bfloat16
    tri = np.triu(np.ones((128, 128), dtype=np.float32)).astype(bf)
    in_maps = []
    for c in range(NC):
        b, hg = c // 2, c % 2
        cols = slice(DH * hg, DH * (hg + 1))
        in_maps.append(
            {
                "xT": np.ascontiguousarray(np.asarray(x)[b].T).astype(bf),
                "wq": np.asarray(Wq)[:, cols].astype(bf),
                "wk": np.asarray(Wk)[:, cols].astype(bf),
                "wv": np.asarray(Wv)[:, cols].astype(bf),
                "wo": np.asarray(Wo)[cols, :].astype(bf),
                "tri": tri,
            }
        )
    res = run_bass_kernel_spmd(nc, in_maps, core_ids=list(range(NC)))
    out = np.empty((B, S, D), dtype=np.float32)
    bo32 = np.asarray(bo, dtype=np.float32)
    for b in range(B):
        acc = res.results[2 * b]["outT"].astype(np.float32) + res.results[2 * b + 1][
            "outT"
        ].astype(np.float32)
        out[b] = acc.T + bo32
    return out


# revision 19
# speedup vs baseline: 1.6253x; 1.3098x over previous
"""Causal multi-head attention (B=4, S=2048, D=1024, H=16, HD=64) on 8 NeuronCores.

Sharding: core c handles batch b=c//2 and head-group hg=c%2 (8 heads each).
Each core computes out^T_partial = Wo_hg^T @ ctx_hg^T for its (b, hg); the host
sums the two head-group partials per batch, transposes, and adds the bias.

All matmuls run in bf16 (fp32 PSUM accumulation); softmax runs without max
subtraction (scores are O(1) here) using exp on the ScalarE and a ones-column
appended to V so the denominators fall out of the ctx matmul for free.
"""

import sys

for _p in ("/opt/trn_rl_repo",):
    if _p not in sys.path:
        sys.path.insert(0, _p)

import numpy as np
import ml_dtypes
from contextlib import ExitStack

import concourse.bacc as bacc
import concourse.tile as tile
from concourse import mybir
from concourse.bass_utils import run_bass_kernel_spmd

F32 = mybir.dt.float32
BF16 = mybir.dt.bfloat16
Exp = mybir.ActivationFunctionType.Exp

B, S, D, H, HD = 4, 2048, 1024, 16, 64
NC = 8          # cores
HL = 8          # heads per core (head-group)
DH = HL * HD    # 512, per-core head dim
KT = D // 128   # 8 k-tiles over d_in
ST = S // 128   # 16 tiles over sequence
NB = S // 512   # 4 q-superblocks
SCALE = 1.0 / np.sqrt(HD)


def _build_nc(debug=False):
    nc = bacc.Bacc("TRN2", target_bir_lowering=False)

    xT = nc.declare_dram_parameter("xT", [D, S], BF16, isOutput=False)
    wq = nc.declare_dram_parameter("wq", [D, DH], BF16, isOutput=False)
    wk = nc.declare_dram_parameter("wk", [D, DH], BF16, isOutput=False)
    wv = nc.declare_dram_parameter("wv", [D, DH], BF16, isOutput=False)
    wo = nc.declare_dram_parameter("wo", [DH, D], BF16, isOutput=False)
    tri = nc.declare_dram_parameter("tri", [128, 128], BF16, isOutput=False)
    outT = nc.declare_dram_parameter("outT", [D, S], F32, isOutput=True)
    if debug:
        d_qT = nc.declare_dram_parameter("d_qT", [DH, S], BF16, isOutput=True)
        d_kT = nc.declare_dram_parameter("d_kT", [DH, S], BF16, isOutput=True)
        d_v = nc.declare_dram_parameter("d_v", [S, HL * (HD + 1)], BF16, isOutput=True)
        d_ctxT = nc.declare_dram_parameter("d_ctxT", [DH, S], BF16, isOutput=True)
        d_cun = nc.declare_dram_parameter("d_cun", [HL * 65, S], F32, isOutput=True)
        d_bcs = nc.declare_dram_parameter("d_bcs", [64, S], BF16, isOutput=True)

    with tile.TileContext(nc) as tc, ExitStack() as ctx:
        const_pool = ctx.enter_context(tc.tile_pool(name="const", bufs=1))
        xT_pool = ctx.enter_context(tc.tile_pool(name="xT", bufs=1))
        w_pool = ctx.enter_context(tc.tile_pool(name="w", bufs=1))
        qk_pool = ctx.enter_context(tc.tile_pool(name="qk", bufs=1))
        v_pool = ctx.enter_context(tc.tile_pool(name="v", bufs=1))
        ctxT_pool = ctx.enter_context(tc.tile_pool(name="ctxT", bufs=1))
        e_pool = ctx.enter_context(tc.tile_pool(name="e", bufs=8))
        r_pool = ctx.enter_context(tc.tile_pool(name="r", bufs=4))
        o_pool = ctx.enter_context(tc.tile_pool(name="o", bufs=2))
        if debug:
            dbg_pool = ctx.enter_context(tc.tile_pool(name="dbg", bufs=1))
        ps_gen = ctx.enter_context(tc.tile_pool(name="ps_gen", bufs=1, space="PSUM"))
        ps_s = ctx.enter_context(tc.tile_pool(name="ps_s", bufs=2, space="PSUM"))
        ps_c = ctx.enter_context(tc.tile_pool(name="ps_c", bufs=1, space="PSUM"))

        # ---- constants ----
        trit = const_pool.tile([128, 128], BF16)
        nc.sync.dma_start(trit[:], tri[:])
        onesb = const_pool.tile([1, 64], BF16)
        nc.vector.memset(onesb[:], 1.0)

        # ---- load x^T and weights ----
        xt = [xT_pool.tile([128, S], BF16, tag=f"xt{_}", name=f"xt{_}") for _ in range(KT)]
        for k in range(KT):
            nc.sync.dma_start(xt[k][:], xT[128 * k : 128 * (k + 1), :])
        wqt = [w_pool.tile([128, DH], BF16, tag=f"wqt{_}", name=f"wqt{_}") for _ in range(KT)]
        wkt = [w_pool.tile([128, DH], BF16, tag=f"wkt{_}", name=f"wkt{_}") for _ in range(KT)]
        wvt = [w_pool.tile([128, DH], BF16, tag=f"wvt{_}", name=f"wvt{_}") for _ in range(KT)]
        for k in range(KT):
            nc.sync.dma_start(wqt[k][:], wq[128 * k : 128 * (k + 1), :])
            nc.sync.dma_start(wkt[k][:], wk[128 * k : 128 * (k + 1), :])
            nc.sync.dma_start(wvt[k][:], wv[128 * k : 128 * (k + 1), :])
        wot = [w_pool.tile([128, D], BF16, tag=f"wot{_}", name=f"wot{_}") for _ in range(DH // 128)]
        for k in range(DH // 128):
            nc.sync.dma_start(wot[k][:], wo[128 * k : 128 * (k + 1), :])

        # ---- phase 1c first: V natural ----
        # ---- phase 1c: V natural [S, DH] as 16 tiles [128, 8*65] (ones col per head) ----
        vt = [v_pool.tile([128, HL * (HD + 1)], BF16, tag=f"v{_}", name=f"v{_}") for _ in range(ST)]
        for st in range(ST):
            nc.vector.memset(vt[st].rearrange("p (h c) -> p h c", c=HD + 1)[:, :, HD], 1.0)
            pv = ps_gen.tile([128, 512], F32, tag=("pgA" if st % 2 == 0 else "pgB"), name="pv")
            for k in range(KT):
                nc.tensor.matmul(
                    pv[:], xt[k][:, 128 * st : 128 * (st + 1)], wvt[k][:],
                    start=(k == 0), stop=(k == KT - 1),
                )
            nc.vector.tensor_copy(
                vt[st].rearrange("p (h c) -> p h c", c=HD + 1)[:, :, 0:HD],
                pv.rearrange("p (h c) -> p h c", c=HD)[:],
            )

        qTt = [qk_pool.tile([128, S], BF16, tag=f"qT{_}", name=f"qT{_}") for _ in range(DH // 128)]
        kTt = [qk_pool.tile([128, S], BF16, tag=f"kT{_}", name=f"kT{_}") for _ in range(DH // 128)]
        def emit_qk_m(m):
            for wt, dst in ((wqt, qTt), (wkt, kTt)):
                for np_ in range(NB // 2):
                    psA = ps_gen.tile([128, 512], F32, tag="pgA", name="psA")
                    psB = ps_gen.tile([128, 512], F32, tag="pgB", name="psB")
                    for k in range(KT):
                        lhsT = wt[k][:, 128 * m : 128 * (m + 1)]
                        for n, pst in ((2 * np_, psA), (2 * np_ + 1, psB)):
                            nc.tensor.matmul(
                                pst[:], lhsT, xt[k][:, 512 * n : 512 * (n + 1)],
                                start=(k == 0), stop=(k == KT - 1),
                            )
                    for n, pst in ((2 * np_, psA), (2 * np_ + 1, psB)):
                        nc.vector.tensor_copy(dst[m][:, 512 * n : 512 * (n + 1)], pst[:])
        # ---- phase 2: per head attention in S^T layout, q-superblock pairs ----
        ctxT = [ctxT_pool.tile([128, S], BF16, tag=f"ctxT{_}", name=f"ctxT{_}") for _ in range(DH // 128)]

        def emit_head(h):
            m, r = h // 2, h % 2
            qTh = qTt[m][64 * r : 64 * r + 64, :]
            kTh = kTt[m][64 * r : 64 * r + 64, :]
            for ip in range(NB // 2):
                I0 = 2 * ip  # superblocks I0, I0+1 handled this sweep
                cps = [
                    ps_c.tile([65, 512], F32, tag=f"pc{_}", name=f"pctx{_}")
                    for _ in range(2)
                ]
                for j in range(8 * ip + 8):
                    jb = j // 4  # first valid superblock for this k-tile
                    # S^T blocks into one [128,1024] psum (2 banks)
                    sp = ps_s.tile([128, 1024], F32, tag="ps", name="sp")
                    e = e_pool.tile([128, 1024], BF16, tag="e", name="e")
                    valid = []  # (li, lo) li: 0/1 slot, lo: masked-out prefix
                    for I in (I0, I0 + 1):
                        if jb > I:
                            continue
                        li = I - I0
                        lo = 128 * (j - 4 * I) if jb == I else 0
                        valid.append((li, lo))
                        nc.tensor.matmul(
                            sp[:, 512 * li + lo : 512 * (li + 1)],
                            kTh[:, 128 * j : 128 * (j + 1)],
                            qTh[:, 512 * I + lo : 512 * (I + 1)],
                            start=True, stop=True,
                        )
                    li0, lo0 = valid[0]
                    base = 512 * li0 + lo0
                    if lo0 > 0:
                        nc.vector.memset(e[:, 512 * li0 : base], 0.0)
                    nc.scalar.activation(
                        e[:, base : 1024], sp[:, base : 1024], Exp, scale=float(SCALE)
                    )
                    for li, lo in valid:
                        if jb == I0 + li:  # diagonal block: triangular mask
                            nc.vector.tensor_tensor(
                                e[:, 512 * li + lo : 512 * li + lo + 128],
                                e[:, 512 * li + lo : 512 * li + lo + 128],
                                trit[:], mybir.AluOpType.mult,
                            )
                    vh = vt[j][:, (HD + 1) * h : (HD + 1) * (h + 1)]
                    for li, lo in valid:
                        I = I0 + li
                        nc.tensor.matmul(
                            cps[li][:], vh, e[:, 512 * li : 512 * (li + 1)],
                            start=(j == 0), stop=(j == 4 * I + 3),
                        )
                    if j % 4 == 3 and jb >= I0:
                        # cps[jb - I0] is complete: normalize it
                        li = jb - I0
                        I = jb
                        cun = r_pool.tile([65, 512], F32, tag="cun", name="cun")
                        nc.vector.tensor_copy(cun[:], cps[li][:])
                        den0 = r_pool.tile([1, 512], F32, tag="den0", name="den0")
                        nc.sync.dma_start(den0[0:1, :], cun[64:65, :])
                        rec0 = r_pool.tile([1, 512], F32, tag="rec0", name="rec0")
                        nc.vector.reciprocal_approx_fast(rec0[0:1, :], den0[0:1, :])
                        recb = r_pool.tile([1, 512], BF16, tag="recb", name="recb")
                        nc.vector.tensor_copy(recb[:], rec0[:])
                        bc = ps_c.tile([65, 512], F32, tag=f"pc{li}", name="bc")[0:64, :]
                        nc.tensor.matmul(
                            bc[:], onesb[0:1, 0:64], recb[0:1, :], start=True, stop=True
                        )
                        nrm = r_pool.tile([64, 512], BF16, tag="nrm", name="nrm")
                        nc.vector.tensor_tensor(
                            nrm[:], cun[0:64, :], bc[:], mybir.AluOpType.mult
                        )
                        nc.sync.dma_start(
                            ctxT[m][64 * r : 64 * r + 64, 512 * I : 512 * (I + 1)],
                            nrm[:],
                        )


        for m in range(DH // 128):
            emit_qk_m(m)
            emit_head(2 * m)
            emit_head(2 * m + 1)

        # ---- phase 3: out^T = Wo^T @ ctx^T  [D, S] ----
        for m in range(D // 128):
            ot = o_pool.tile([128, S], F32, tag="ot")
            for np_ in range(NB // 2):
                psA = ps_gen.tile([128, 512], F32, tag="pgA", name="poA")
                psB = ps_gen.tile([128, 512], F32, tag="pgB", name="poB")
                for k in range(DH // 128):
                    lhsT = wot[k][:, 128 * m : 128 * (m + 1)]
                    for n, pst in ((2 * np_, psA), (2 * np_ + 1, psB)):
                        nc.tensor.matmul(
                            pst[:], lhsT, ctxT[k][:, 512 * n : 512 * (n + 1)],
                            start=(k == 0), stop=(k == DH // 128 - 1),
                        )
                for n, pst in ((2 * np_, psA), (2 * np_ + 1, psB)):
                    nc.vector.tensor_copy(ot[:, 512 * n : 512 * (n + 1)], pst[:])
            nc.sync.dma_start(outT[128 * m : 128 * (m + 1), :], ot[:])

    nc.compile()
    return nc


_NC_CACHE = None


def kernel(x, Wq, Wk, Wv, Wo, bo):
    global _NC_CACHE
    if _NC_CACHE is None:
        _NC_CACHE = _build_nc()
    nc = _NC_CACHE

    bf = ml_dtypes.bfloat16
    tri = np.triu(np.ones((128, 128), dtype=np.float32)).astype(bf)
    in_maps = []
    for c in range(NC):
        b, hg = c // 2, c % 2
        cols = slice(DH * hg, DH * (hg + 1))
        in_maps.append(
            {
                "xT": np.ascontiguousarray(np.asarray(x)[b].T).astype(bf),
                "wq": np.asarray(Wq)[:, cols].astype(bf),
                "wk": np.asarray(Wk)[:, cols].astype(bf),
                "wv": np.asarray(Wv)[:, cols].astype(bf),
                "wo": np.asarray(Wo)[cols, :].astype(bf),
                "tri": tri,
            }
        )
    res = run_bass_kernel_spmd(nc, in_maps, core_ids=list(range(NC)))
    out = np.empty((B, S, D), dtype=np.float32)
    bo32 = np.asarray(bo, dtype=np.float32)
    for b in range(B):
        acc = res.results[2 * b]["outT"].astype(np.float32) + res.results[2 * b + 1][
            "outT"
        ].astype(np.float32)
        out[b] = acc.T + bo32
    return out
